# revision 14
# baseline (speedup 1.0000x reference)
"""Trainium2 Bass kernel for nn_BigramBaseline: causal mean pooling over
embedding-gathered rows.

  logits[b*T + t, :] = mean_{s<=t} emb[idx[b, s], :]

Strategy (data-parallel over batch, one batch row per core):
  - per 128-token block: indirect-DMA gather of 128 emb rows -> SBUF
    tile [128, V] (partition = token within block)
  - in-block causal prefix sum via PE matmul with a lower-triangular
    ones matrix (lhsT = upper-triangular incl. diag)
  - cross-block carry kept resident in PSUM: after emitting the block's
    prefix sums, a second matmul with the strict complement mask adds
    the rest of the block's column-sums, turning the PSUM bank into
    carry_{k+1} broadcast over all 128 partitions
  - scale by 1/(t+1) during the PSUM->SBUF copy on the scalar engine
    (per-partition scale operand), then DMA out
"""

import os

import numpy as np

B, T, V = 8, 2048, 4096
P = 128
CHUNK = 512
N_CORES = 8

USE_F32R = os.environ.get("BIGRAM_F32R", "1") == "1"


def build_bass(t=T, v=V, use_f32r=USE_F32R):
    import concourse.bacc as bacc
    import concourse.bass as bass
    import concourse.tile as tile
    from concourse import mybir

    nblk = t // P
    chunk = min(CHUNK, v)
    nchunk = v // chunk

    # float32r: same 4-byte fp32 payload, but tags the PE-bound data path so
    # the fast fp32 matmul mode (1 cycle/row vs 4) passes BIR verification.
    mm_dt = mybir.dt.float32r if use_f32r else mybir.dt.float32

    # Bacc (not plain Bass): its finalize() runs generate_event_semaphores,
    # which splits multi-sem waits — walrus codegen only fits one sync wait
    # per instruction.
    nc = bacc.Bacc(trn_type="TRN2")
    emb = nc.declare_dram_parameter("emb", [v, v], mm_dt, isOutput=False)
    idx = nc.declare_dram_parameter("idx", [P, nblk], mybir.dt.int32, isOutput=False)
    invd = nc.declare_dram_parameter("invd", [P, nblk], mybir.dt.float32, isOutput=False)
    # masks[:, 0:P]  = lhsT for the in-block prefix sum: m[s, p] = 1 iff s <= p
    # masks[:, P:2P] = lhsT for the carry update:        m[s, p] = 1 iff s > p
    masks = nc.declare_dram_parameter("masks", [P, 2 * P], mm_dt, isOutput=False)
    out = nc.declare_dram_parameter("out", [t, v], mybir.dt.float32, isOutput=True)

    with tile.TileContext(nc) as tc:
        with (
            tc.tile_pool(name="const", bufs=1) as cpool,
            tc.tile_pool(name="x", bufs=3) as xpool,
            tc.tile_pool(name="o", bufs=3) as opool,
            tc.tile_pool(name="acc", bufs=1, space="PSUM") as ppool,
        ):
            idx_sb = cpool.tile([P, nblk], mybir.dt.int32)
            nc.sync.dma_start(out=idx_sb[:], in_=idx[:])
            invd_sb = cpool.tile([P, nblk], mybir.dt.float32)
            nc.sync.dma_start(out=invd_sb[:], in_=invd[:])
            masks_sb = cpool.tile([P, 2 * P], mm_dt)
            nc.sync.dma_start(out=masks_sb[:], in_=masks[:])
            trilT_sb = masks_sb[:, 0:P]
            strictT_sb = masks_sb[:, P : 2 * P]

            acc = [
                ppool.tile([P, chunk], mybir.dt.float32, name=f"acc{c}", tag=f"acc{c}")
                for c in range(nchunk)
            ]

            # Walrus only fits ONE sync wait per engine instruction, so each
            # engine pre-absorbs its constant-DMA wait in a tiny warm-up op;
            # the real ops then carry only their single data-flow wait.
            nc.tensor.matmul(
                out=acc[0][:, 0:2],
                lhsT=trilT_sb,
                rhs=masks_sb[:, 0:2],
                start=True,
                stop=True,
                skip_group_check=True,
            )
            scratch = cpool.tile([P, 1], mybir.dt.float32)
            nc.scalar.activation(
                out=scratch[:],
                in_=invd_sb[:, 0:1],
                func=mybir.ActivationFunctionType.Copy,
            )

            for k in range(nblk):
                x = xpool.tile([P, v], mm_dt)
                nc.gpsimd.indirect_dma_start(
                    out=x[:],
                    out_offset=None,
                    in_=emb[:],
                    in_offset=bass.IndirectOffsetOnAxis(
                        ap=idx_sb[:, k : k + 1], axis=0
                    ),
                )
                o = opool.tile([P, v], mybir.dt.float32)
                for c in range(nchunk):
                    sl = bass.ts(c, chunk)
                    nc.tensor.matmul(
                        out=acc[c][:],
                        lhsT=trilT_sb,
                        rhs=x[:, sl],
                        start=(k == 0),
                        stop=True,
                        skip_group_check=True,
                    )
                    nc.scalar.activation(
                        out=o[:, sl],
                        in_=acc[c][:],
                        func=mybir.ActivationFunctionType.Copy,
                        scale=invd_sb[:, k : k + 1],
                    )
                    if k < nblk - 1:
                        nc.tensor.matmul(
                            out=acc[c][:],
                            lhsT=strictT_sb,
                            rhs=x[:, sl],
                            start=False,
                            stop=True,
                            skip_group_check=True,
                        )
                nc.sync.dma_start(out=out[bass.ts(k, P), :], in_=o[:])
                # Dead write into the just-shipped tile: routes the output
                # DMA's completion through the ACT engine, so the pool-slot
                # reuse 3 blocks later costs the next activation no extra
                # sync wait (1-wait-per-instruction limit).
                nc.scalar.activation(
                    out=o[:, 0:1],
                    in_=invd_sb[:, 0:1],
                    func=mybir.ActivationFunctionType.Copy,
                )
    nc.finalize()
    return nc


def host_inputs(idx_row, emb, t=T, v=V):
    """Per-core input map for one batch row. idx_row: [t] int, emb: [v, v] f32."""
    nblk = t // P
    idx32 = np.ascontiguousarray(
        np.asarray(idx_row, dtype=np.int32).reshape(nblk, P).T
    )
    invd = np.ascontiguousarray(
        (1.0 / np.arange(1, t + 1, dtype=np.float64))
        .astype(np.float32)
        .reshape(nblk, P)
        .T
    )
    masks = np.concatenate(
        [
            np.triu(np.ones((P, P), dtype=np.float32)),
            np.tril(np.ones((P, P), dtype=np.float32), -1),
        ],
        axis=1,
    )
    return {
        "emb": np.ascontiguousarray(np.asarray(emb, dtype=np.float32)),
        "idx": idx32,
        "invd": invd,
        "masks": np.ascontiguousarray(masks),
    }


_nc_cache = {}


def kernel(idx, emb, _trace=False):
    from concourse.bass_utils import run_bass_kernel_spmd

    key = "nc"
    if key not in _nc_cache:
        _nc_cache[key] = build_bass()
    nc = _nc_cache[key]

    idx = np.asarray(idx)
    emb_np = np.ascontiguousarray(np.asarray(emb, dtype=np.float32))
    in_maps = [host_inputs(idx[b], emb_np) for b in range(N_CORES)]
    res = run_bass_kernel_spmd(nc, in_maps, list(range(N_CORES)), trace=_trace)
    kernel.last_results = res
    out = np.concatenate([r["out"] for r in res.results], axis=0)
    return out
